# revision 1
# baseline (speedup 1.0000x reference)
"""Multi-head attention (B=4, N=2048, C=1024, H=16) on 8 TRN2 NeuronCores.

Sharding: core c = (batch b = c//2, head-group hg = c%2), 8 heads per group.
Each core computes its head-group's attention for its batch plus the partial
output projection against the matching w_out rows; the host sums the two
partials per batch and adds the bias terms (exact: softmax rows sum to 1, so
the v-bias contributes b_v @ w_out + b_out as a constant row).

Device pipeline (per core), all matmuls bf16 (inputs pre-cast on host):
  1. v token-major with a fused ones column per head (the ones column makes
     the PV matmul accumulate the softmax denominator in psum row 64 free)
  2. per head-pair g: q/k projections channel-major, then attention --
     scores S^T[nk,nq] as two tile_position-packed K=64 matmuls, exp on
     ScalarE straight out of psum ([128,1024] per op, scale 1/8 folded in),
     PV accumulation over nk, then normalize via reciprocal + PE broadcast.
     Emission order interleaves pair g+1's projections under pair g's
     ACT-bound attention.
  3. output projection token-major, streamed to HBM
"""

import numpy as np

B, N, C = 4, 2048, 1024
H, Dh = 16, 64
HG = 8  # heads per core
P = 128
KK = C // P       # 8 contraction tiles for the projections
NT = N // P       # 16 token/nk tiles
NQ = N // 512     # 4 query chunks

_CACHE = {}


def _build():
    import concourse.bass as bass
    import concourse.tile as tile
    from concourse import mybir, bacc
    from contextlib import ExitStack

    f32 = mybir.dt.float32
    f32r = mybir.dt.float32r
    bf16 = mybir.dt.bfloat16
    FT = mybir.ActivationFunctionType
    OP = mybir.AluOpType

    nc = bacc.Bacc("TRN2", target_bir_lowering=False, debug=False)

    xT = nc.dram_tensor("xT", [C, N], bf16, kind="ExternalInput").ap()
    wq = nc.dram_tensor("wq", [C, 512], bf16, kind="ExternalInput").ap()
    wk = nc.dram_tensor("wk", [C, 512], bf16, kind="ExternalInput").ap()
    wv = nc.dram_tensor("wv", [C, 512], bf16, kind="ExternalInput").ap()
    bqk = nc.dram_tensor("bqk", [P, 8], f32, kind="ExternalInput").ap()
    wo = nc.dram_tensor("wo", [512, C], bf16, kind="ExternalInput").ap()
    out = nc.dram_tensor("out", [N, C], f32, kind="ExternalOutput").ap()

    def r(ap):
        return ap.bitcast(f32r)

    with tile.TileContext(nc) as tc, ExitStack() as ctx, \
         nc.allow_low_precision(reason="bf16 attention pipeline"):
        pool = lambda name, bufs: ctx.enter_context(
            tc.tile_pool(name=name, bufs=bufs))
        qkT_pool = pool("qkT", 1)
        v_pool = pool("v", 1)
        attT_pool = pool("attT", 1)
        const_pool = pool("const", 1)
        x_pool = pool("x", 1)
        w_pool = pool("w", 1)
        exp_pool = pool("expst", 15)
        ou_pool = pool("ou", 6)
        rp_pool = pool("rp", 3)
        wo_pool = pool("wo", 1)
        out_pool = pool("outst", 2)
        pscore = ctx.enter_context(
            tc.tile_pool(name="pscore", bufs=2, space="PSUM"))
        ppv = ctx.enter_context(tc.tile_pool(name="ppv", bufs=2, space="PSUM"))
        pfill = ctx.enter_context(tc.tile_pool(name="pfill", bufs=2, space="PSUM"))

        qkT = [qkT_pool.tile([P, N], bf16, tag=f"qkT{i}", name=f"qkT{i}")
               for i in range(8)]
        vt = [v_pool.tile([P, HG * 65], bf16, tag=f"v{i}", name=f"vt{i}")
              for i in range(NT)]
        attT = [attT_pool.tile([P, N], bf16, tag=f"attT{i}", name=f"attT{i}")
                for i in range(4)]

        ones_f32 = const_pool.tile([1, 64], f32, tag="ones32", name="ones_f32")
        nc.vector.memset(ones_f32[:], 1.0)
        ones_t = const_pool.tile([1, 64], f32r, tag="ones", name="ones_t")
        nc.vector.tensor_copy(ones_t[:], ones_f32[:])
        biasqk_raw = const_pool.tile([P, 8], f32, tag="bqkr", name="biasqk_raw")
        nc.sync.dma_start(biasqk_raw[:], bqk)
        biasqk = const_pool.tile([P, 8], f32, tag="bqk", name="biasqk")
        nc.vector.tensor_copy(biasqk[:], biasqk_raw[:])

        # resident inputs (all bf16, pre-cast on host). DMAs spread across
        # the SP + ACT HWDGE queues and the gpsimd SWDGE queue so the
        # startup load is parallel, q/k weights + x first.
        ET = mybir.EngineType
        qeng = [nc.sync, nc.scalar, nc.gpsimd]

        def load(ap, name, qi):
            return x_pool.tile_from(ap, name=name)

        xt = [load(xT[kk * P:(kk + 1) * P, :], f"xt{kk}", kk)
              for kk in range(KK)]
        wqk_t = [load(wq[kk * P:(kk + 1) * P, :], f"wqt{kk}", kk)
                 for kk in range(KK)]
        wqk_t += [load(wk[kk * P:(kk + 1) * P, :], f"wkt{kk}", kk + 1)
                  for kk in range(KK)]
        wv_t = [load(wv[kk * P:(kk + 1) * P, :], f"wvt{kk}", kk)
                for kk in range(KK)]
        wo_t = [load(wo[kk * P:(kk + 1) * P, :], f"wot{kk}", kk)
                for kk in range(4)]

        def qk_group(mt, j):
            ps = pfill.tile([P, 512], f32, tag="pf", name="psa")
            for kk in range(KK):
                w_ap = wqk_t[(mt // 4) * KK + kk][:, (mt % 4) * P:
                                                  (mt % 4 + 1) * P]
                nc.tensor.matmul(ps[:], w_ap,
                                 xt[kk][:, j * 512:(j + 1) * 512],
                                 start=(kk == 0), stop=(kk == KK - 1))
            nc.vector.tensor_scalar_add(
                qkT[mt][:, j * 512:(j + 1) * 512], ps[:],
                biasqk[:, mt:mt + 1])

        def v_group(mg):
            ps = pfill.tile([P, 512], f32, tag="pf", name="psa")
            for kk in range(KK):
                nc.tensor.matmul(ps[:], xt[kk][:, mg * P:(mg + 1) * P],
                                 wv_t[kk][:],
                                 start=(kk == 0), stop=(kk == KK - 1))
            vg = vt[mg][:].rearrange("p (h c) -> p h c", c=65)
            nc.vector.tensor_copy(vg[:, :, 0:64],
                                  ps[:].rearrange("p (h c) -> p h c", c=64))
            nc.vector.memset(vg[:, :, 64:65], 1.0)

        def attention_head(h, fillers, inline_v=False):
            qT_h = qkT[h // 2][(h % 2) * 64:(h % 2) * 64 + 64, :]
            kT_h = qkT[4 + h // 2][(h % 2) * 64:(h % 2) * 64 + 64, :]
            nfill = len(fillers)
            fi = 0
            D = 6  # scores/exp run D steps ahead of PV
            po_sets = {}
            es = {}

            def scores_exp(s):
                jh, t = s // NT, s % NT
                if t == 0:
                    po_sets[jh] = [ppv.tile([65, 512], f32, tag="po",
                                            name=f"po{i}") for i in range(2)]
                e = exp_pool.tile([P, 1024], bf16, tag="e", name="et")
                ps = pscore.tile([P, 1024], f32, tag="sc", name="psc")
                for jj in range(2):
                    j = 2 * jh + jj
                    nc.tensor.matmul(ps[:, jj * 512:(jj + 1) * 512],
                                     kT_h[:, t * P:(t + 1) * P],
                                     qT_h[:, j * 512:(j + 1) * 512],
                                     start=True, stop=True)
                nc.scalar.activation(e[:], ps[:], FT.Exp, scale=Dh ** -0.5)
                es[s] = e

            def normalize(jh):
                po = po_sets.pop(jh)
                ocp = []
                for jj in range(2):
                    o = ou_pool.tile([65, 512], f32, tag="o", name="otile")
                    nc.vector.tensor_copy(o[:], po[jj][:])
                    ocp.append(o)
                for jj in range(2):
                    j = 2 * jh + jj
                    o = ocp[jj]
                    rec = rp_pool.tile([1, 512], f32r, tag="r", name="rtile")
                    with nc.allow_low_precision(reason="softmax denom"):
                        nc.vector.reciprocal(rec[:], o[64:65, :])
                    pb = pfill.tile([64, 512], f32, tag="pf", name="pb")
                    nc.tensor.matmul(pb[:], ones_t[:].bitcast(f32r),
                                     rec[:].bitcast(f32r),
                                     start=True, stop=True)
                    nc.vector.tensor_tensor(
                        attT[h // 2][(h % 2) * 64:(h % 2) * 64 + 64,
                                     j * 512:(j + 1) * 512],
                        o[0:64, :], pb[:], op=OP.mult)

            def pv(s):
                jh, t = s // NT, s % NT
                e = es.pop(s)
                if inline_v and jh == 0:
                    v_group(t)
                for jj in range(2):
                    nc.tensor.matmul(po_sets[jh][jj][:],
                                     vt[t][:, h * 65:h * 65 + 65],
                                     e[:, jj * 512:(jj + 1) * 512],
                                     start=(t == 0), stop=(t == NT - 1))
                if t == NT - 1:
                    normalize(jh)

            for s in range(2 * NT + D):
                if s < 2 * NT:
                    scores_exp(s)
                if s >= D:
                    pv(s - D)
                if s < 2 * NT:
                    while fi < nfill and fi < ((s + 1) * nfill) // (2 * NT):
                        fillers[fi]()
                        fi += 1

        # pair-0 projections first; v production inlined into head 0's
        # first half; later pairs' projections spread as fillers
        for mt in (0, 4):
            for j in range(NQ):
                qk_group(mt, j)
        for h in range(HG):
            fillers = []
            if h in (1, 3, 5):
                g = h // 2 + 1
                fillers = [
                    (lambda mt=mt, j=j: qk_group(mt, j))
                    for mt in (g, 4 + g) for j in range(NQ)]
            attention_head(h, fillers, inline_v=(h == 0))
                # ---- output projection, token-major ------------------------------
        for m in range(NT):
            ob = out_pool.tile([P, C], f32, tag="ob", name="ob")
            for c in range(2):
                cpool = pfill if (m + c) % 2 == 0 else ppv
                ctag = "pf" if (m + c) % 2 == 0 else "po"
                ps = cpool.tile([P, 512], f32, tag=ctag, name="psa")
                for kk in range(4):
                    nc.tensor.matmul(ps[:],
                                     attT[kk][:, m * P:(m + 1) * P],
                                     wo_t[kk][:, c * 512:(c + 1) * 512],
                                     start=(kk == 0), stop=(kk == 3))
                nc.vector.tensor_copy(ob[:, c * 512:(c + 1) * 512], ps[:])
            nc.sync.dma_start(out[m * P:(m + 1) * P, :], ob[:])

    nc.compile()
    return nc


def _in_maps(x, w_qkv, b_qkv, w_out):
    import ml_dtypes
    bf = ml_dtypes.bfloat16
    x = np.asarray(x, np.float32)
    w_qkv = np.asarray(w_qkv, np.float32)
    b_qkv = np.asarray(b_qkv, np.float32)
    w_out = np.asarray(w_out, np.float32)
    maps = []
    for core in range(8):
        b, hg = core // 2, core % 2
        s = slice(hg * 512, hg * 512 + 512)
        maps.append({
            "xT": np.ascontiguousarray(x[b].T).astype(bf),
            "wq": np.ascontiguousarray(w_qkv[:, 0 * C:1 * C][:, s]).astype(bf),
            "wk": np.ascontiguousarray(w_qkv[:, 1 * C:2 * C][:, s]).astype(bf),
            "wv": np.ascontiguousarray(w_qkv[:, 2 * C:3 * C][:, s]).astype(bf),
            "bqk": np.ascontiguousarray(np.concatenate(
                [b_qkv[0 * C:1 * C][s], b_qkv[1 * C:2 * C][s]])
                .reshape(8, P).T),
            "wo": np.ascontiguousarray(w_out[s, :]).astype(bf),
        })
    return maps


def _gather(results, b_qkv, b_out, w_out):
    out = np.zeros((B, N, C), np.float32)
    for core in range(8):
        out[core // 2] += np.asarray(results[core]["out"], np.float32)
    # exact bias terms: softmax rows sum to 1, so +b_v contributes b_v @ w_out
    out += (np.asarray(b_qkv[2 * C:3 * C], np.float32)
            @ np.asarray(w_out, np.float32) + np.asarray(b_out, np.float32))
    return out


def run(x, w_qkv, b_qkv, w_out, b_out, trace=False):
    from concourse.bass_utils import run_bass_kernel_spmd
    if "nc" not in _CACHE:
        _CACHE["nc"] = _build()
    res = run_bass_kernel_spmd(_CACHE["nc"], _in_maps(x, w_qkv, b_qkv, w_out),
                               list(range(8)), trace=trace)
    _CACHE["last_res"] = res
    return _gather(res.results, b_qkv, b_out, w_out), res.exec_time_ns


def kernel(x, w_qkv, b_qkv, w_out, b_out):
    out, _ = run(x, w_qkv, b_qkv, w_out, b_out)
    return out



# revision 13
# speedup vs baseline: 1.6955x; 1.6955x over previous
"""Multi-head attention (B=4, N=2048, C=1024, H=16) on 8 TRN2 NeuronCores.

Sharding: core c = (batch b = c//2, head-group hg = c%2), 8 heads per group.
Each core computes its head-group's attention for its batch plus the partial
output projection against the matching w_out rows; the host sums the two
partials per batch and adds the bias terms (exact: softmax rows sum to 1, so
the v-bias contributes b_v @ w_out + b_out as a constant row).

Device pipeline (per core), all matmuls bf16 (inputs pre-cast on host):
  1. v token-major with a fused ones column per head (the ones column makes
     the PV matmul accumulate the softmax denominator in psum row 64 free)
  2. heads processed in PAIRS (A at partitions 0-63, B at 64-127 of the same
     qkT tile): per step one [128,1024] psum score tile is filled by two
     K=64 matmuls on PE row-tiles T0/T8 (concurrent in 64x128 tiling mode),
     one ScalarE exp covers both heads, then two K=128 PV matmuls
     accumulate po_A/po_B. Softmax normalization per 512-token query chunk
     uses reciprocal_approx_fast + PE ones-broadcast.
  3. output projection token-major, streamed to HBM
"""

import numpy as np

B, N, C = 4, 2048, 1024
H, Dh = 16, 64
HG = 8  # heads per core
P = 128
KK = C // P       # 8 contraction tiles for the projections
NT = N // P       # 16 nk tiles
NQ4 = 4           # 512-token query chunks

_CACHE = {}


def _build():
    import concourse.bass as bass
    import concourse.tile as tile
    from concourse import mybir, bacc
    from contextlib import ExitStack

    f32 = mybir.dt.float32
    f32r = mybir.dt.float32r
    bf16 = mybir.dt.bfloat16
    FT = mybir.ActivationFunctionType
    OP = mybir.AluOpType

    nc = bacc.Bacc("TRN2", target_bir_lowering=False, debug=False)

    xT = nc.dram_tensor("xT", [C, N], bf16, kind="ExternalInput").ap()
    wq = nc.dram_tensor("wq", [C, 512], bf16, kind="ExternalInput").ap()
    wk = nc.dram_tensor("wk", [C, 512], bf16, kind="ExternalInput").ap()
    wv = nc.dram_tensor("wv", [C, 512], bf16, kind="ExternalInput").ap()
    bqk = nc.dram_tensor("bqk", [P, 8], f32, kind="ExternalInput").ap()
    wo = nc.dram_tensor("wo", [512, C], bf16, kind="ExternalInput").ap()
    selc = nc.dram_tensor("selc", [8, 512], f32, kind="ExternalInput").ap()
    out = nc.dram_tensor("out", [N, C], f32, kind="ExternalOutput").ap()

    with tile.TileContext(nc) as tc, ExitStack() as ctx, \
         nc.allow_low_precision(reason="bf16 attention pipeline"):
        pool = lambda name, bufs: ctx.enter_context(
            tc.tile_pool(name=name, bufs=bufs))
        qkT_pool = pool("qkT", 1)
        v_pool = pool("v", 1)
        attT_pool = pool("attT", 1)
        const_pool = pool("const", 1)
        x_pool = pool("x", 1)
        exp_pool = pool("expst", 10)
        ou_pool = pool("ou", 8)
        rp_pool = pool("rp", 3)
        out_pool = pool("outst", 2)
        pscore = ctx.enter_context(
            tc.tile_pool(name="pscore", bufs=2, space="PSUM"))
        ppo = ctx.enter_context(tc.tile_pool(name="ppo", bufs=2, space="PSUM"))
        pfill = ctx.enter_context(
            tc.tile_pool(name="pfill", bufs=2, space="PSUM"))

        qkT = [qkT_pool.tile([P, N], bf16, tag=f"qkT{i}", name=f"qkT{i}")
               for i in range(8)]
        vt = [v_pool.tile([P, HG * 65], bf16, tag=f"v{i}", name=f"vt{i}")
              for i in range(NT)]
        attT = [attT_pool.tile([P, N], bf16, tag=f"attT{i}", name=f"attT{i}")
                for i in range(4)]

        ones_f32 = const_pool.tile([1, 64], f32, tag="ones32", name="ones_f32")
        nc.vector.memset(ones_f32[:], 1.0)
        ones_t = const_pool.tile([1, 64], f32r, tag="ones", name="ones_t")
        nc.vector.tensor_copy(ones_t[:], ones_f32[:])
        # selector weights: sel[:, i*64:(i+1)*64] is [8,64] with row i ones,
        # used to broadcast row i of an [8,512] tile via a K=8 matmul
        sel_f32 = const_pool.tile([8, 512], f32, tag="sel32", name="sel_f32")
        nc.sync.dma_start(sel_f32[:], selc)
        sel_t = const_pool.tile([8, 512], f32r, tag="sel", name="sel_t")
        nc.vector.tensor_copy(sel_t[:], sel_f32[:])
        biasqk_raw = const_pool.tile([P, 8], f32, tag="bqkr", name="biasqk_raw")
        nc.sync.dma_start(biasqk_raw[:], bqk)
        biasqk = const_pool.tile([P, 8], f32, tag="bqk", name="biasqk")
        nc.vector.tensor_copy(biasqk[:], biasqk_raw[:])

        def load(ap, name):
            return x_pool.tile_from(ap, name=name)

        xt = [load(xT[kk * P:(kk + 1) * P, :], f"xt{kk}") for kk in range(KK)]
        wqk_t = [load(wq[kk * P:(kk + 1) * P, :], f"wqt{kk}")
                 for kk in range(KK)]
        wqk_t += [load(wk[kk * P:(kk + 1) * P, :], f"wkt{kk}")
                  for kk in range(KK)]
        wv_t = [load(wv[kk * P:(kk + 1) * P, :], f"wvt{kk}")
                for kk in range(KK)]
        wo_t = [load(wo[kk * P:(kk + 1) * P, :], f"wot{kk}") for kk in range(4)]

        def qk_group(mt, j):
            # q (mt 0-3) / k (mt 4-7) projection: heads 2*(mt%4), 2*(mt%4)+1
            ps = pfill.tile([P, 512], f32, tag="pf", name="psa")
            for kk in range(KK):
                w_ap = wqk_t[(mt // 4) * KK + kk][:, (mt % 4) * P:
                                                  (mt % 4 + 1) * P]
                nc.tensor.matmul(ps[:], w_ap,
                                 xt[kk][:, j * 512:(j + 1) * 512],
                                 start=(kk == 0), stop=(kk == KK - 1))
            nc.vector.tensor_scalar_add(
                qkT[mt][:, j * 512:(j + 1) * 512], ps[:],
                biasqk[:, mt:mt + 1])

        def v_group(mg):
            ps = pfill.tile([P, 512], f32, tag="pf", name="psa")
            for kk in range(KK):
                nc.tensor.matmul(ps[:], xt[kk][:, mg * P:(mg + 1) * P],
                                 wv_t[kk][:],
                                 start=(kk == 0), stop=(kk == KK - 1))
            vg = vt[mg][:].rearrange("p (h c) -> p h c", c=65)
            nc.vector.tensor_copy(vg[:, :, 0:64],
                                  ps[:].rearrange("p (h c) -> p h c", c=64))
            nc.vector.memset(vg[:, :, 64:65], 1.0)

        def attention_pair(g, fillers, inline_v=False):
            hA, hB = 2 * g, 2 * g + 1
            qA = qkT[g][0:64, :]
            qB = qkT[g][64:128, :]
            kA = qkT[4 + g][0:64, :]
            kB = qkT[4 + g][64:128, :]
            nfill = len(fillers)
            fi = 0
            D = 4  # scores/exp run D steps ahead of PV
            es = {}
            po = {}
            otiles = []  # (off, q, o) accumulated over quarters

            total = NQ4 * NT  # 64 steps
            for s in range(total + D):
                if s < total:
                    q, t = divmod(s, NT)
                    ps = pscore.tile([P, 1024], f32, tag="sc", name="psc")
                    nc.tensor.matmul(ps[:, 0:512],
                                     kA[:, t * P:(t + 1) * P],
                                     qA[:, q * 512:(q + 1) * 512],
                                     start=True, stop=True)
                    nc.tensor.matmul(ps[:, 512:1024],
                                     kB[:, t * P:(t + 1) * P],
                                     qB[:, q * 512:(q + 1) * 512],
                                     start=True, stop=True)
                    e = exp_pool.tile([P, 1024], bf16, tag="e", name="et")
                    nc.scalar.activation(e[:], ps[:], FT.Exp, scale=Dh ** -0.5)
                    es[s] = e
                if s >= D:
                    s2 = s - D
                    q2, t2 = divmod(s2, NT)
                    if t2 == 0:
                        po["A"] = ppo.tile([65, 512], f32, tag="po",
                                           name="poA")
                        po["B"] = ppo.tile([65, 512], f32, tag="po",
                                           name="poB")
                    e2 = es.pop(s2)
                    if inline_v and q2 == 0:
                        v_group(t2)
                    nc.tensor.matmul(po["A"][:],
                                     vt[t2][:, hA * 65:hA * 65 + 65],
                                     e2[:, 0:512],
                                     start=(t2 == 0), stop=(t2 == NT - 1))
                    nc.tensor.matmul(po["B"][:],
                                     vt[t2][:, hB * 65:hB * 65 + 65],
                                     e2[:, 512:1024],
                                     start=(t2 == 0), stop=(t2 == NT - 1))
                    if t2 == NT - 1:
                        for X, off in (("A", 0), ("B", 64)):
                            p = po.pop(X)
                            o = ou_pool.tile([65, 512], f32, tag="o",
                                             name="otile")
                            nc.vector.tensor_copy(o[:], p[:])
                            otiles.append((off, q2, o))
                while fi < nfill and fi < ((s + 1) * nfill) // (total + D):
                    fillers[fi]()
                    fi += 1

            # deferred normalize: one batched reciprocal for all 8 chunks
            # (gather den rows to partitions 0-7 via DMA: engine APs cannot
            # start at unaligned partitions, DMA can place anywhere)
            dd = rp_pool.tile([8, 512], f32, tag="dd", name="ddt")
            for i, (off, q, o) in enumerate(otiles):
                nc.sync.dma_start(dd[i:i + 1, :], o[64:65, :])
            rr = rp_pool.tile([8, 512], f32r, tag="rr", name="rrt")
            with nc.allow_low_precision(reason="softmax denom"):
                nc.vector.reciprocal(rr[:], dd[:])
            for i, (off, q, o) in enumerate(otiles):
                pbt = pfill.tile([64, 512], f32, tag="pf", name="pb")
                nc.tensor.matmul(pbt[:], sel_t[:, i * 64:(i + 1) * 64],
                                 rr[:], start=True, stop=True)
                nc.vector.tensor_tensor(
                    attT[g][off:off + 64, q * 512:(q + 1) * 512],
                    o[0:64, :], pbt[:], op=OP.mult)

        # pair-0 projections first; v production inlined into pair 0's
        # first query chunk; later pairs' projections spread as fillers
        for mt in (0, 4):
            for j in range(NQ4):
                qk_group(mt, j)
        for g in range(4):
            fillers = []
            if g < 3:
                fillers = [
                    (lambda mt=mt, j=j: qk_group(mt, j))
                    for mt in (g + 1, 4 + g + 1) for j in range(NQ4)]
            attention_pair(g, fillers, inline_v=(g == 0))

        # ---- output projection, token-major ------------------------------
        for m in range(NT):
            ob = out_pool.tile([P, C], f32, tag="ob", name="ob")
            ps = pscore.tile([P, 1024], f32, tag="sc", name="psc")
            for c in range(2):
                for kk in range(4):
                    nc.tensor.matmul(ps[:, c * 512:(c + 1) * 512],
                                     attT[kk][:, m * P:(m + 1) * P],
                                     wo_t[kk][:, c * 512:(c + 1) * 512],
                                     start=(kk == 0), stop=(kk == 3))
            nc.vector.tensor_copy(ob[:], ps[:])
            nc.sync.dma_start(out[m * P:(m + 1) * P, :], ob[:])

    nc.compile()
    return nc


def _in_maps(x, w_qkv, b_qkv, w_out):
    import ml_dtypes
    bf = ml_dtypes.bfloat16
    x = np.asarray(x, np.float32)
    w_qkv = np.asarray(w_qkv, np.float32)
    b_qkv = np.asarray(b_qkv, np.float32)
    w_out = np.asarray(w_out, np.float32)
    maps = []
    for core in range(8):
        b, hg = core // 2, core % 2
        s = slice(hg * 512, hg * 512 + 512)
        maps.append({
            "xT": np.ascontiguousarray(x[b].T).astype(bf),
            "wq": np.ascontiguousarray(w_qkv[:, 0 * C:1 * C][:, s]).astype(bf),
            "wk": np.ascontiguousarray(w_qkv[:, 1 * C:2 * C][:, s]).astype(bf),
            "wv": np.ascontiguousarray(w_qkv[:, 2 * C:3 * C][:, s]).astype(bf),
            "bqk": np.ascontiguousarray(np.concatenate(
                [b_qkv[0 * C:1 * C][s], b_qkv[1 * C:2 * C][s]])
                .reshape(8, P).T),
            "wo": np.ascontiguousarray(w_out[s, :]).astype(bf),
            "selc": np.ascontiguousarray(
                np.repeat(np.eye(8, dtype=np.float32), 64, axis=1)),
        })
    return maps


def _gather(results, b_qkv, b_out, w_out):
    out = np.zeros((B, N, C), np.float32)
    for core in range(8):
        out[core // 2] += np.asarray(results[core]["out"], np.float32)
    # exact bias terms: softmax rows sum to 1, so +b_v contributes b_v @ w_out
    out += (np.asarray(b_qkv[2 * C:3 * C], np.float32)
            @ np.asarray(w_out, np.float32) + np.asarray(b_out, np.float32))
    return out


def run(x, w_qkv, b_qkv, w_out, b_out, trace=False):
    from concourse.bass_utils import run_bass_kernel_spmd
    if "nc" not in _CACHE:
        _CACHE["nc"] = _build()
    res = run_bass_kernel_spmd(_CACHE["nc"], _in_maps(x, w_qkv, b_qkv, w_out),
                               list(range(8)), trace=trace)
    _CACHE["last_res"] = res
    return _gather(res.results, b_qkv, b_out, w_out), res.exec_time_ns


def kernel(x, w_qkv, b_qkv, w_out, b_out):
    out, _ = run(x, w_qkv, b_qkv, w_out, b_out)
    return out
